# revision 27
# baseline (speedup 1.0000x reference)
"""Trainium2 Bass kernel for nn_BlockV3 (dense transformer block).

Sharding: 8 cores = 2 (batch) x 4 (query-quarter). Each core holds the full
batch element for K/V and computes attention + MLP for its own 512 query
rows. Host-side prep reorders tokens per core (own 512 first) so the device
program is identical across cores (SPMD), and pre-transposes / pre-blocks /
casts the weights so the device kernel is fully feature-major with zero
on-chip transposes.

Folding done on host (exact fp32 algebra):
  - LN gains/biases fold into the following linear: W' = W*g, b' = W@b_ln + b
  - V-projection bias folds through attention (rows of att sum to 1) into the
    out-projection bias: bp'' = bp + Wp@bv'
  - the padding/cond mask is folded into V (masked rows zeroed; a per-head
    65th column carries the mask) so exp runs unmasked and the softmax
    denominator is recovered in the att@V matmul.

Perf structure (vs the naive emission):
  - LN rstd via a single Rsqrt activation (no Ln/Exp table thrash).
  - Softmax denominators are collected per pair and processed in ONE batched
    Rsqrt after the attention loop (1/d = rsqrt(d)^2), so the attention hp
    loop has no scalar-engine table switches and no PE stalls.
  - Score exps are batched: both heads of a pair go into one [128,1024]
    2-bank PSUM tile and one FD=1024 exp drains it.
  - V projection and both MLP linears run fp8 DoubleRow (K=256 per matmul),
    halving their matmul counts.
"""

import sys
import numpy as np

sys.path.insert(0, "/opt/trn_rl_repo")

B = 2
T = 2048
C = 768
H = 12
Dh = 64
F = 3072
P = 128
NCH = C // P          # 6 feature chunks
NCP = NCH // 2        # 3 feature chunk-pairs (fp8 DoubleRow)
NFT = F // P          # 24 mlp chunks
NFP = NFT // 2        # 12 mlp chunk-pairs
NKT = T // P          # 16 key tiles
TQ = 512              # own query rows per core
NQ4 = T // TQ         # 4 t-quarters
N_CORES = 8
EPS = 1e-5

_CACHE = {}


def _build_nc():
    import concourse.bass as bass
    from concourse import bacc, mybir
    import concourse.tile as tile

    f32 = mybir.dt.float32
    bf16 = mybir.dt.bfloat16

    nc = bacc.Bacc()
    eps_t = nc.alloc_sbuf_tensor("const-eps", [128, 1], f32)
    nc.gpsimd.memset(eps_t.ap(), EPS)
    nc.const_aps.aps[(f32, EPS)] = eps_t.ap()

    f8 = mybir.dt.float8e4
    d = {}
    d["xT"] = nc.declare_dram_parameter("xT", [C, T], bf16, isOutput=False)
    d["xTown"] = nc.declare_dram_parameter("xTown", [C, TQ], bf16, isOutput=False)
    d["mbias"] = nc.declare_dram_parameter("mbias", [T], f32, isOutput=False)
    d["wqB"] = nc.declare_dram_parameter("wqB", [NCH, P, NCP, 2, P], f8, isOutput=False)
    d["wkB"] = nc.declare_dram_parameter("wkB", [NCH, P, NCP, 2, P], f8, isOutput=False)
    d["wvR"] = nc.declare_dram_parameter("wvR", [NCH, P, C], f8, isOutput=False)
    d["wpB"] = nc.declare_dram_parameter("wpB", [NCH, P, NCH, P], bf16, isOutput=False)
    d["w1B"] = nc.declare_dram_parameter("w1B", [NFT, P, NCH, P], bf16, isOutput=False)
    d["w2B"] = nc.declare_dram_parameter("w2B", [NCH, P, NFT, P], bf16, isOutput=False)
    d["bqR"] = nc.declare_dram_parameter("bqR", [P, NCH], f32, isOutput=False)
    d["bkR"] = nc.declare_dram_parameter("bkR", [P, NCH], f32, isOutput=False)
    d["boR"] = nc.declare_dram_parameter("boR", [P, NCH], f32, isOutput=False)
    d["b1R"] = nc.declare_dram_parameter("b1R", [P, NFT], f32, isOutput=False)
    d["b2R"] = nc.declare_dram_parameter("b2R", [P, NCH], f32, isOutput=False)
    d["sel"] = nc.declare_dram_parameter("sel", [2, P], bf16, isOutput=False)
    d["outT"] = nc.declare_dram_parameter("outT", [C, TQ], f32, isOutput=True)

    with tile.TileContext(nc) as tc:
        _emit(tc, nc, mybir, bass, tile, d)
    nc.finalize()
    return nc


def _emit(tc, nc, mybir, bass, tile, g):
    from contextlib import ExitStack

    f32 = mybir.dt.float32
    bf16 = mybir.dt.bfloat16
    f8 = mybir.dt.float8e4
    AF = mybir.ActivationFunctionType
    OP = mybir.AluOpType
    DR = mybir.MatmulPerfMode.DoubleRow
    ts = bass.ts
    ds = bass.ds

    xT, xTown, mbias = g["xT"], g["xTown"], g["mbias"]
    wqB, wkB, wvR, wpB, w1B, w2B = (g["wqB"], g["wkB"], g["wvR"], g["wpB"],
                                    g["w1B"], g["w2B"])
    bqR, bkR, boR, b1R, b2R, selD, outT = (
        g["bqR"], g["bkR"], g["boR"], g["b1R"], g["b2R"], g["sel"], g["outT"])

    ctx = ExitStack()
    with ctx:
        psum = ctx.enter_context(tc.tile_pool(name="psum", bufs=4, space="PSUM"))
        sb = ctx.enter_context(tc.tile_pool(name="sb", bufs=1))

        def pt1(name):
            # single-bank psum tile [P, TQ]
            return psum.tile([P, TQ], f32, tag="mm", bufs=4, name=name)

        def pt2(name):
            # two-bank psum tile [P, 2*TQ]
            return psum.tile([P, 2 * TQ], f32, tag="sp", bufs=2, name=name)

        def st(shape, dtype, tag, bufs, name):
            return sb.tile(shape, dtype, tag=tag, bufs=bufs, name=name)

        # ---- constants / small loads ----
        mb = st([P, NKT], f32, "mb", 1, "mb")
        nc.sync.dma_start(mb, mbias[:].rearrange("(c p) -> p c", p=P))
        bq_s = st([P, NCH], f32, "bq", 1, "bq_s")
        nc.sync.dma_start(bq_s, bqR[:, :])
        bk_s = st([P, NCH], f32, "bk", 1, "bk_s")
        nc.sync.dma_start(bk_s, bkR[:, :])
        bo_s = st([P, NCH], f32, "bo", 1, "bo_s")
        nc.sync.dma_start(bo_s, boR[:, :])
        b1_s = st([P, NFT], f32, "b1", 1, "b1_s")
        nc.sync.dma_start(b1_s, b1R[:, :])
        b2_s = st([P, NCH], f32, "b2", 1, "b2_s")
        nc.sync.dma_start(b2_s, b2R[:, :])
        sel_s = st([2, P], bf16, "sel", 1, "sel_s")
        nc.sync.dma_start(sel_s, selD[:, :])
        # per-token-sum weights: ones * (1/C) so the stat matmuls emit means
        ones_c = st([P, 1], bf16, "ones_c", 1, "ones_c")
        nc.vector.memset(ones_c, 1.0 / C)
        # broadcast rows: +1 block (for a) and -1 block (for b = -mu*a)
        ones_blk = st([P, P], f32, "ones_blk", 1, "ones_blk")
        nc.vector.memset(ones_blk, 1.0)
        nones_blk = st([P, P], f32, "nones_blk", 1, "nones_blk")
        nc.vector.memset(nones_blk, -1.0)

        # ================= Phase 1: LN1 statistics over full T =================
        # s1/s2 quarters live at partition offsets {0,64} of 2x2 banks.
        sb1 = [pt1("s1b0"), pt1("s1b1")]
        sb2 = [pt1("s2b0"), pt1("s2b1")]

        def qrow(q):
            # quarters 0-2 at partitions {0,32,64} of bank 0; quarter 3 at
            # partition 0 of bank 1 (rust caps base_partition at 64)
            if q < 3:
                return 0, slice(32 * q, 32 * q + 1)
            return 1, slice(0, 1)

        for c in range(NCH):
            xt = st([P, T], bf16, "big", 4, f"xt1_{c}")
            (nc.sync if c % 2 == 0 else nc.scalar).dma_start(
                xt, xT[c * P:(c + 1) * P, :])
            xsq = st([P, T], bf16, "big", 4, f"xsq{c}")
            nc.vector.tensor_tensor(xsq, xt, xt, OP.mult)
            for q in range(NQ4):
                qi, r0 = qrow(q)
                nc.tensor.matmul(sb1[qi][r0, :], ones_c,
                                 xt[:, ts(q, TQ)], start=(c == 0),
                                 stop=(c == NCH - 1),
                                 tile_position=(0, 32 * q if q < 3 else 0),
                                 skip_group_check=True)
                nc.tensor.matmul(sb2[qi][r0, :], ones_c,
                                 xsq[:, ts(q, TQ)], start=(c == 0),
                                 stop=(c == NCH - 1),
                                 tile_position=(0, 32 * q if q < 3 else 0),
                                 skip_group_check=True)
        # Scratch rows at partitions {0,64} of two tiles. rstd =
        # exp(-0.5*ln(var+eps)); the Ln ops then the Exp ops are grouped by
        # function so each table set loads exactly once, and Exp stays
        # resident entering the attention loop.
        mu_t = [st([P, TQ], f32, "rows", 4, f"mu_t{i}") for i in range(2)]
        var_t = [st([P, TQ], f32, "rows", 4, f"var_t{i}") for i in range(2)]
        arow_t = var_t
        a_full = st([P, T], bf16, "abf", 2, "a_full")
        b_full = st([P, T], bf16, "abf", 2, "b_full")
        # batched row math: one [65,512] op covers quarters 0-2 (partitions
        # 0/32/64; rows between are junk lanes at no extra cost), plus one
        # [1,512] op for quarter 3 in bank 1.
        spans = [(0, slice(0, 65)), (1, slice(0, 1))]
        for qi, rs in spans:
            nc.vector.tensor_copy(mu_t[qi][rs, :], sb1[qi][rs, :])
            nc.vector.tensor_tensor(var_t[qi][rs, :], mu_t[qi][rs, :],
                                    mu_t[qi][rs, :], OP.mult)
            nc.vector.tensor_tensor(var_t[qi][rs, :], sb2[qi][rs, :],
                                    var_t[qi][rs, :], OP.subtract)
        for qi, rs in spans:
            nc.scalar.activation(arow_t[qi][rs, :], var_t[qi][rs, :], AF.Ln,
                                 bias=EPS, scale=1.0)
        for qi, rs in spans:
            nc.scalar.activation(arow_t[qi][rs, :], arow_t[qi][rs, :], AF.Exp,
                                 bias=0.0, scale=-0.5)
        for qi, rs in spans:
            nc.vector.tensor_tensor(mu_t[qi][rs, :], mu_t[qi][rs, :],
                                    arow_t[qi][rs, :], OP.mult)
        for q in range(NQ4):
            qi, r0 = qrow(q)
            pa = pt1(f"pa{q}")
            nc.tensor.matmul(pa, ones_blk[r0, :], arow_t[qi][r0, :],
                             start=True, stop=True)
            nc.scalar.copy(a_full[:, ts(q, TQ)], pa)
            pb = pt1(f"pb{q}")
            nc.tensor.matmul(pb, nones_blk[r0, :], mu_t[qi][r0, :],
                             start=True, stop=True)
            nc.vector.tensor_copy(b_full[:, ts(q, TQ)], pb)

        # ================= Phase 2: LN1 apply, straight to f8 =================
        u1f8 = st([P, NCH, T], f8, "u1f8", 1, "u1f8")
        for c in range(NCH):
            xt = st([P, T], bf16, "big", 4, f"xt2_{c}")
            (nc.sync if c % 2 == 0 else nc.scalar).dma_start(
                xt, xT[c * P:(c + 1) * P, :])
            u = st([P, T], bf16, "big", 4, f"u1_{c}")
            nc.vector.tensor_tensor(u, xt, a_full, OP.mult)
            nc.vector.tensor_tensor(u, u, b_full, OP.add)
            nc.scalar.copy(u1f8[:, c, :], u)

        # ---- fused QKV + attention emission ----
        ystack = [st([P, TQ], bf16, "yst", NCH, f"ystack{i}") for i in range(NCH)]

        # Q projection: feature-major q^T [C, TQ] (own rows only), bf16
        qt = []
        for ot in range(NCH):
            wq = st([P, NCP, 2, P], f8, "wkq", 4, f"wq{ot}")
            nc.sync.dma_start(wq, wqB[ot])
            qp = pt1(f"qp{ot}")
            for kp2 in range(NCP):
                nc.tensor.matmul(qp, wq[:, kp2, :, :],
                                 u1f8[:, 2 * kp2:2 * kp2 + 2, 0:TQ],
                                 start=(kp2 == 0), stop=(kp2 == NCP - 1),
                                 perf_mode=DR)
            qs = st([P, TQ], bf16, "qu", NCH, f"qt{ot}")
            nc.vector.tensor_scalar_add(qs, qp, bq_s[:, ot:ot + 1])
            qt.append(qs)

        # K projection pieces: feature-major k^T [C, T] (full batch element)
        kt = []
        wks = []
        for ot in range(NCH):
            kt.append(st([P, T], bf16, "kt", NCH, f"kt{ot}"))
            wks.append(None)

        def emit_k_weight(ot):
            w = st([P, NCP, 2, P], f8, "wkq", 4, f"wk{ot}")
            nc.sync.dma_start(w, wkB[ot])
            wks[ot] = w

        def emit_k_quarter(ot, gq):
            kp = pt1(f"kp{ot}_{gq}")
            for kp2 in range(NCP):
                nc.tensor.matmul(kp, wks[ot][:, kp2, :, :],
                                 u1f8[:, 2 * kp2:2 * kp2 + 2, ts(gq, TQ)],
                                 start=(kp2 == 0), stop=(kp2 == NCP - 1),
                                 perf_mode=DR)
            nc.vector.tensor_scalar_add(kt[ot][:, ts(gq, TQ)], kp,
                                        bk_s[:, ot:ot + 1])

        # V projection: token-major v [T, C], fp8 DoubleRow (K=256 per mm),
        # with the 0/1 mask folded in: masked rows zeroed, per-head 65th
        # column = mask.
        wv_s = st([P, NCH, C], f8, "wv", 1, "wv_s")
        for kc in range(NCH):
            nc.sync.dma_start(wv_s[:, kc, :], wvR[kc])
        vt = [None] * (NKT // 2)

        def emit_v_tile(tk):
            vp = pt2(f"vp{tk}")
            va = vp[:, 0:TQ]
            vb = vp[:, TQ:TQ + 256]
            for kp2 in range(NCP):
                lhs = u1f8[:, 2 * kp2:2 * kp2 + 2, ts(tk, P)]
                nc.tensor.matmul(va, lhs, wv_s[:, 2 * kp2:2 * kp2 + 2, 0:512],
                                 start=(kp2 == 0), stop=(kp2 == NCP - 1),
                                 perf_mode=DR)
                nc.tensor.matmul(vb, lhs, wv_s[:, 2 * kp2:2 * kp2 + 2, 512:768],
                                 start=(kp2 == 0), stop=(kp2 == NCP - 1),
                                 perf_mode=DR)
            if tk % 2 == 0:
                vt[tk // 2] = st([P, 2, H, 68], f8, "vp", NKT // 2,
                                 f"v{tk // 2}")
            v = vt[tk // 2][:, tk % 2, :, :]
            va3 = va.rearrange("p (h d) -> p h d", d=64)
            vb3 = vb.rearrange("p (h d) -> p h d", d=64)
            mcol = mb[:, tk:tk + 1]
            nc.vector.tensor_scalar_mul(v[:, 0:8, 0:64], va3, mcol)
            nc.vector.tensor_scalar_mul(v[:, 8:12, 0:64], vb3, mcol)
            nc.vector.tensor_copy(v[:, :, 64:65], mcol.to_broadcast((P, H, 1)))

        # softmax denominators, collected pair-by-pair: [12, TQ] bf16
        den2 = st([2 * NCH, TQ], bf16, "den", 1, "den2")

        def finish_pair(hp, yp):
            """Evacuate the two yas psum banks; defer the 1/den scaling."""
            for h2 in range(2):
                h = 2 * hp + h2
                rows = slice(64 * h2, 64 * h2 + 64)
                yc = st([65, TQ], bf16, "yc", 3, f"yc{h}")
                nc.vector.tensor_copy(yc, yp[h2])
                # cross-partition moves go through SBUF->SBUF DMA
                nc.sync.dma_start(ystack[hp][rows, :], yc[0:64, :])
                nc.sync.dma_start(den2[h:h + 1, :], yc[64:65, :])

        emit_k_weight(0)
        emit_k_quarter(0, 0)
        emit_k_quarter(0, 1)
        emit_k_quarter(0, 2)
        emit_k_quarter(0, 3)
        prev_et = None
        prev_yas = None
        for hp in range(NCH):
            # et tiles: [P, 2(j=tk parity), 2(h2), TQ] f8, one per gp
            et = [st([P, 2, 2, TQ], f8, "et", 12, f"et{hp}_{gp}")
                  for gp in range(NKT // 2)]
            if hp >= 1:
                yas = [pt1(f"ya{2 * (hp - 1) + h2}")[0:65, :] for h2 in range(2)]
            if hp <= NCH - 2:
                emit_k_weight(hp + 1)
            for tk in range(NKT):
                gp, j = tk // 2, tk % 2
                sp2 = pt2(f"sp{hp}_{tk}")
                # both heads of the pair; the two matmuls run concurrently
                # (row groups 0-1 vs 2-3), targeting the tile's two banks
                for h2 in range(2):
                    rows = slice(64 * h2, 64 * h2 + 64)
                    nc.tensor.matmul(sp2[:, ts(h2, TQ)],
                                     kt[hp][rows, ts(tk, P)],
                                     qt[hp][rows, :], start=True, stop=True)
                # one batched exp over both heads (FD=1024, 2 banks)
                nc.scalar.activation(et[gp][:, j, :, :], sp2, AF.Exp,
                                     bias=0.0, scale=0.125)
                if hp == 0 and tk < 8:
                    emit_v_tile(tk)
                if hp == 1 and tk < 8:
                    emit_v_tile(8 + tk)
                if hp >= 1 and j == 1:
                    for h2 in range(2):
                        nc.tensor.matmul(
                            yas[h2],
                            vt[gp][:, :, 2 * (hp - 1) + h2, 0:65],
                            prev_et[gp][:, :, h2, :],
                            start=(gp == 0), stop=(gp == NKT // 2 - 1),
                            perf_mode=DR)
                if hp <= NCH - 2 and tk % 4 == 3:
                    emit_k_quarter(hp + 1, tk // 4)
            if hp >= 1:
                finish_pair(hp - 1, yas)
            prev_et = et
        yas = [pt1(f"ya{2 * (NCH - 1) + h2}")[0:65, :] for h2 in range(2)]
        for gp in range(NKT // 2):
            for h2 in range(2):
                nc.tensor.matmul(
                    yas[h2], vt[gp][:, :, 2 * (NCH - 1) + h2, 0:65],
                    prev_et[gp][:, :, h2, :],
                    start=(gp == 0), stop=(gp == NKT // 2 - 1),
                    perf_mode=DR)
        finish_pair(NCH - 1, yas)

        # ---- batched denominator: 1/d on the vector engine (no ACT table) ----
        den_f = st([2 * NCH, TQ], f32, "denf", 2, "den_f")
        nc.vector.tensor_copy(den_f, den2)
        den_i = st([2 * NCH, TQ], f32, "denf", 2, "den_i")
        nc.vector.reciprocal_approx_fast(den_i, den_f)
        den_b = st([2 * NCH, TQ], bf16, "denb", 1, "den_b")
        nc.vector.tensor_copy(den_b, den_i)

        for hp in range(NCH):
            # matmul operands must sit at partition 0: hoist the pair rows
            dp = st([2, TQ], bf16, "denp", 2, f"dp{hp}")
            nc.sync.dma_start(dp, den_b[2 * hp:2 * hp + 2, :])
            rp = pt1(f"rp{hp}")
            nc.tensor.matmul(rp, sel_s, dp, start=True, stop=True)
            rb = st([P, TQ], bf16, "rb", 2, f"rb{hp}")
            nc.vector.tensor_copy(rb, rp)
            nc.vector.tensor_tensor(ystack[hp], ystack[hp], rb, OP.mult)

        # ================= Phase 4: out-projection + residual =================
        xb2 = []
        for ot in range(NCH):
            wp = st([P, NCH, P], bf16, "w15", 2, f"wp{ot}")
            nc.sync.dma_start(wp, wpB[ot])
            xp = pt1(f"xp{ot}")
            for kc in range(NCH):
                nc.tensor.matmul(xp, wp[:, kc, :], ystack[kc],
                                 start=(kc == 0), stop=(kc == NCH - 1))
            xo = st([P, TQ], bf16, "xtown", 2, f"xo{ot}")
            nc.sync.dma_start(xo, xTown[ot * P:(ot + 1) * P, :])
            xb = st([P, TQ], bf16, "xb2", NCH, f"xb2_{ot}")
            nc.vector.tensor_scalar_add(xb, xp, bo_s[:, ot:ot + 1])
            nc.vector.tensor_tensor(xb, xb, xo, OP.add)
            xb2.append(xb)

        # ================= Phase 5: LN2 (own rows, bf16 stats) ================
        s1p2 = pt1("s1p2")[0:1, :]
        s2p2 = pt1("s2p2")[0:1, :]
        for c in range(NCH):
            xsq2 = st([P, TQ], bf16, "xsq2", 1, f"xsq2_{c}")
            nc.vector.tensor_tensor(xsq2, xb2[c], xb2[c], OP.mult)
            nc.tensor.matmul(s1p2, ones_c, xb2[c], start=(c == 0),
                             stop=(c == NCH - 1))
            nc.tensor.matmul(s2p2, ones_c, xsq2, start=(c == 0),
                             stop=(c == NCH - 1))
        r2 = st([1, TQ], f32, "r2", 3, "r2_var")
        a2r = st([1, TQ], f32, "r2", 3, "r2_a")
        mu2 = st([1, TQ], f32, "r2", 3, "r2_mu")
        nc.vector.tensor_copy(mu2, s1p2)
        nc.vector.tensor_tensor(r2, mu2, mu2, OP.mult)
        nc.vector.tensor_tensor(r2, s2p2, r2, OP.subtract)
        nc.scalar.activation(a2r, r2, AF.Ln, bias=EPS, scale=1.0)
        nc.scalar.activation(a2r, a2r, AF.Exp, bias=0.0, scale=-0.5)
        nc.vector.tensor_tensor(r2, mu2, a2r, OP.mult)
        pa2 = pt1("pa2")
        nc.tensor.matmul(pa2, ones_blk[0:1, :], a2r, start=True, stop=True)
        a2b = st([P, TQ], bf16, "ab2", 2, "a2b")
        nc.vector.tensor_copy(a2b, pa2)
        pb2 = pt1("pb2")
        nc.tensor.matmul(pb2, nones_blk[0:1, :], r2, start=True, stop=True)
        b2b = st([P, TQ], bf16, "ab2", 2, "b2b")
        nc.vector.tensor_copy(b2b, pb2)
        u2 = []
        for c in range(NCH):
            u = st([P, TQ], bf16, "qu", NCH, f"u2_{c}")
            nc.vector.tensor_tensor(u, xb2[c], a2b, OP.mult)
            nc.vector.tensor_tensor(u, u, b2b, OP.add)
            u2.append(u)

        # ================= Phase 6: MLP (bf16 for accuracy) =================
        gt = []
        for mt in range(NFT):
            w1 = st([P, NCH, P], bf16, "w1t", 6, f"w1_{mt}")
            nc.sync.dma_start(w1, w1B[mt])
            mp = pt1(f"mp{mt}")
            for kc in range(NCH):
                nc.tensor.matmul(mp, w1[:, kc, :], u2[kc],
                                 start=(kc == 0), stop=(kc == NCH - 1))
            gs = st([P, TQ], bf16, "ysgt", NFT, f"gt{mt}")
            nc.scalar.activation(gs, mp, AF.Gelu, bias=b1_s[:, mt:mt + 1],
                                 scale=1.0)
            gt.append(gs)
        for ot in range(NCH):
            w2a = st([P, NFT // 2, P], bf16, "w2st", 4, f"w2a{ot}")
            nc.sync.dma_start(w2a, w2B[ot, :, 0:NFT // 2, :])
            w2b = st([P, NFT // 2, P], bf16, "w2st", 4, f"w2b{ot}")
            nc.sync.dma_start(w2b, w2B[ot, :, NFT // 2:NFT, :])
            op_ = pt1(f"op{ot}")
            for kc in range(NFT):
                wsl = w2a[:, kc, :] if kc < NFT // 2 else w2b[:, kc - NFT // 2, :]
                nc.tensor.matmul(op_, wsl, gt[kc],
                                 start=(kc == 0), stop=(kc == NFT - 1))
            ot_s = st([P, TQ], f32, "outt", 2, f"ot{ot}")
            nc.vector.tensor_scalar_add(ot_s, op_, b2_s[:, ot:ot + 1])
            nc.vector.tensor_tensor(ot_s, ot_s, xb2[ot], OP.add)
            nc.sync.dma_start(outT[ot * P:(ot + 1) * P, :], ot_s)


def _get_nc():
    if "nc" not in _CACHE:
        _CACHE["nc"] = _build_nc()
    return _CACHE["nc"]


def _host_prep(inputs):
    import ml_dtypes
    bf = ml_dtypes.bfloat16
    f8 = ml_dtypes.float8_e4m3fn

    x = np.asarray(inputs["x"], np.float32)
    cond_len = int(np.asarray(inputs["cond_len"]))
    pm = np.asarray(inputs["padding_mask"])
    g1 = np.asarray(inputs["g1"], np.float32)
    bln1 = np.asarray(inputs["bln1"], np.float32)
    g2 = np.asarray(inputs["g2"], np.float32)
    bln2 = np.asarray(inputs["bln2"], np.float32)
    Wq = np.asarray(inputs["Wq"], np.float32)
    Wk = np.asarray(inputs["Wk"], np.float32)
    Wv = np.asarray(inputs["Wv"], np.float32)
    Wp = np.asarray(inputs["Wp"], np.float32)
    W1 = np.asarray(inputs["W1"], np.float32)
    W2 = np.asarray(inputs["W2"], np.float32)
    bq = np.asarray(inputs["bq"], np.float32)
    bk = np.asarray(inputs["bk"], np.float32)
    bv = np.asarray(inputs["bv"], np.float32)
    bp = np.asarray(inputs["bp"], np.float32)
    b1 = np.asarray(inputs["b1"], np.float32)
    b2 = np.asarray(inputs["b2"], np.float32)

    Wq_ = Wq * g1[None, :]
    Wk_ = Wk * g1[None, :]
    Wv_ = Wv * g1[None, :]
    bq_ = Wq @ bln1 + bq
    bk_ = Wk @ bln1 + bk
    bv_ = Wv @ bln1 + bv
    bp_ = bp + Wp @ bv_
    W1_ = W1 * g2[None, :]
    b1_ = W1 @ bln2 + b1

    def blk(WT):
        # WT [K, M] -> [M/128, 128(kp), K/128, 128(m)]
        Kd, Md = WT.shape
        return np.ascontiguousarray(
            WT.reshape(Kd // P, P, Md // P, P).transpose(2, 1, 0, 3)).astype(bf)

    def blk_dr(WT):
        # WT [K, M] -> [M/128, 128(k), K/256, 2(j), 128(m)] for DoubleRow
        Kd, Md = WT.shape
        return np.ascontiguousarray(
            WT.reshape(Kd // 256, 2, P, Md // P, P).transpose(3, 2, 0, 1, 4)
        ).astype(f8)

    def bre(b):
        return np.ascontiguousarray(b.reshape(-1, P).T).astype(np.float32)

    sel = np.zeros((2, P), bf)
    sel[0, 0:Dh] = 1.0
    sel[1, Dh:2 * Dh] = 1.0

    n_b = T - pm.sum(axis=1)
    cols = np.arange(T)
    allowed = (cols[None, :] >= cond_len) | (cols[None, :] < np.asarray(n_b)[:, None])
    M = allowed.astype(np.float32)

    shared = dict(
        wqB=blk_dr(Wq_.T), wkB=blk_dr(Wk_.T),
        wvR=np.ascontiguousarray(Wv_.T.reshape(NCH, P, C)).astype(f8),
        wpB=blk(Wp.T), w1B=blk(W1_.T), w2B=blk(W2.T),
        bqR=bre(bq_), bkR=bre(bk_), boR=bre(bp_), b1R=bre(b1_), b2R=bre(b2),
        sel=sel)

    in_maps = []
    perms = []
    for core in range(N_CORES):
        b = core // 4
        qi = core % 4
        own = np.arange(qi * TQ, (qi + 1) * TQ)
        rest = np.concatenate([np.arange(0, qi * TQ), np.arange((qi + 1) * TQ, T)])
        perm = np.concatenate([own, rest])
        perms.append((b, qi))
        xb = x[b]
        m = dict(shared)
        m.update(
            xT=np.ascontiguousarray(xb[perm].T).astype(bf),
            xTown=np.ascontiguousarray(xb[own].T).astype(bf),
            mbias=np.ascontiguousarray(M[b][perm]))
        in_maps.append(m)
    return in_maps, perms


def kernel(**inputs):
    from concourse.bass_utils import run_bass_kernel_spmd

    nc = _get_nc()
    in_maps, perms = _host_prep(inputs)
    res = run_bass_kernel_spmd(nc, in_maps, list(range(N_CORES)),
                               **_CACHE.get("run_kwargs", {}))
    _CACHE["last_results"] = res
    x = np.asarray(inputs["x"])
    out = np.zeros((B, T, C), np.float32)
    for core in range(N_CORES):
        b, qi = perms[core]
        out[b, qi * TQ:(qi + 1) * TQ, :] = res.results[core]["outT"].T
    return out.astype(x.dtype)
